# revision 10
# baseline (speedup 1.0000x reference)
"""Distributed Trainium2 Bass kernel for the associative-embedding (AE) loss.

Problem: per image b (B=8), two tag maps (tm0 [J,256,256], tm1 [J,512,512]),
keypoints kps [NH, 3*J] (x, y, vis interleaved, NH=30 humans, J=17 joints).
Per level: gather tag values at (j, x, y), masked per-human mean, pull loss
(masked squared deviation / num_humans) + push loss (pairwise Gaussian of
means / num_humans^2).  Output: per-image loss [B] (sum over both levels).

Strategy: pure data-parallel over B across 8 NeuronCores (core b handles
image b).  The loss touches only the ~NH*2*J visible-keypoint elements of
the 22 MB of tag maps.  SWDGE descriptor generation costs ~1 us fixed per
instruction, so the gather is split by level to minimize instruction count:

* level 0 (map is 1.1M floats): flat//64 fits int16, so ONE custom
  dma_gather instruction fetches a 64-float (256 B) block around every
  visible level-0 keypoint (k -> out[k%128, k//128, :]); a host-baked
  one-hot (DVE multiply + segmented reduce) extracts the target element of
  each block while the level-1 gathers are still in flight.
* level 1 (4.5M floats, индex too big for int16): ceil(V1/128) classic
  indirect DMAs, one 4-byte descriptor per element, small remainder chunk
  last so its data drains quickly.

Host-baked one-hot matrices let the tensor engine reduce the packed slot
layout into per-human sufficient statistics via lhsT = E*S, rhs =
[L0, L1, S*L0, S*L1]; chunks are processed while later gathers are still
in flight.  The push loss uses a 32x32 DVE stream transpose, two DVE ops
for the squared pairwise differences, one Exp on the scalar engine, a
ones-vector matmul, and a 2-op weighted reduce; all input-only quantities
(masks, reciprocal counts, 1/num_humans weights) are host-precomputed.
Per-core output is one scalar; the host stacks the 8 scalars into [B].
"""

import numpy as np

B = 8
NH = 30
J = 17
H0 = W0 = 256
H1 = W1 = 512
N0 = J * H0 * W0
N1 = J * H1 * W1
NTOT = N0 + N1
EB = 64                   # dma_gather block: 64 f32 = 256 B
BIG = 30.0                # pad rows -> exp(-(BIG+avg)^2/2) ~ 1e-170 ~ 0

_CACHE = {}


def _cdiv(a, b):
    return -(-a // b)


# ---------------------------------------------------------------------------
# host-side input prep: valid-packed gather indices + one-hot reduction maps
# ---------------------------------------------------------------------------


def make_in_maps(tag_maps0, tag_maps1, kps0, kps1):
    tag_maps0 = np.asarray(tag_maps0, dtype=np.float32)
    tag_maps1 = np.asarray(tag_maps1, dtype=np.float32)
    kps0 = np.asarray(kps0, dtype=np.int64)
    kps1 = np.asarray(kps1, dtype=np.int64)
    jr = np.arange(J)[None, :]
    per_img = []
    n0_max = n1_max = 0
    for b in range(B):
        xs0, ys0, vs0 = kps0[b, :, 0::3], kps0[b, :, 1::3], kps0[b, :, 2::3]
        xs1, ys1, vs1 = kps1[b, :, 0::3], kps1[b, :, 1::3], kps1[b, :, 2::3]
        flat0 = jr * (H0 * W0) + xs0 * W0 + ys0          # [NH, J] local lvl0
        flat1 = N0 + jr * (H1 * W1) + xs1 * W1 + ys1     # [NH, J] global lvl1
        m0 = vs0 != 0
        m1 = vs1 != 0
        h0, j0 = np.nonzero(m0)
        h1, j1 = np.nonzero(m1)
        per_img.append((flat0[h0, j0], h0, flat1[h1, j1], h1, m0, m1))
        n0_max = max(n0_max, len(h0))
        n1_max = max(n1_max, len(h1))
    NCH0 = _cdiv(n0_max, 128)         # dma_gather chunks (level 0)
    NCH1 = _cdiv(n1_max, 128)         # indirect rounds (level 1)
    n1last = n1_max - 128 * (NCH1 - 1)
    NCT = NCH0 + NCH1
    N16 = NCH0 * 128                  # baked dma_gather num_idxs

    in_maps = []
    for b in range(B):
        f0, h0, f1, h1, m0, m1 = per_img[b]
        n0, n1 = len(f0), len(f1)
        # level-0: block indices (int16, replicated over the 8 Q7 cores) and
        # in-block offsets; pad slots keep idx 0 / one-hot 0
        idx16 = np.zeros(N16, np.int16)
        idx16[:n0] = f0 // EB
        ki16 = np.tile(
            np.ascontiguousarray(idx16.reshape(N16 // 16, 16).T), (8, 1)
        )  # [128, N16//16]
        OH = np.zeros((128, NCH0 * EB), np.float32)
        k0 = np.arange(n0)
        OH[k0 % 128, (k0 // 128) * EB + f0 % EB] = 1.0
        # level-1: classic per-element indirect rounds
        kiL1 = np.zeros((128, NCH1), np.int32)
        k1 = np.arange(n1)
        kiL1[k1 % 128, k1 // 128] = f1
        # one-hot reduction maps over the NCT chunk-slot layout
        E = np.zeros((128, NCT * NH), np.float32)
        T = np.zeros((128, 4 * NCT), np.float32)
        E[k0 % 128, (k0 // 128) * NH + h0] = 1.0
        T[k0 % 128, (k0 // 128) * 4 + 0] = 1.0
        E[k1 % 128, (NCH0 + k1 // 128) * NH + h1] = 1.0
        T[k1 % 128, (NCH0 + k1 // 128) * 4 + 1] = 1.0

        cnt = np.stack([m0.sum(1), m1.sum(1)], 1).astype(np.float32)
        has = (cnt > 0).astype(np.float32)
        rdh = has / np.maximum(cnt, 1.0)
        P = 1.0 / has.sum(0)          # [2] 1/num_humans per level
        # kf layout [128, NCH0*EB + 4*NCT + NCT*NH + 8]:
        #   cols 0:OHW                OH extraction one-hot
        #   next 4*NCT                T/rhs region (host: L0,L1,0,0; DVE: S*L)
        #   next NCT*NH               E one-hot
        #   next 2: -rdh | next 2: rdh | next 4 (row 0): w4
        OHW = NCH0 * EB
        kf = np.zeros((128, OHW + 4 * NCT + NCT * NH + 8), np.float32)
        kf[:, 0:OHW] = OH
        kf[:, OHW : OHW + 4 * NCT] = T
        kf[:, OHW + 4 * NCT : OHW + 4 * NCT + NCT * NH] = E
        base = OHW + 4 * NCT + NCT * NH
        kf[0:NH, base : base + 2] = -rdh
        kf[0:NH, base + 2 : base + 4] = rdh
        kf[0, base + 4 : base + 8] = [P[0] ** 2, P[1] ** 2, P[0], P[1]]
        tm = np.concatenate(
            [tag_maps0[b].ravel(), tag_maps1[b].ravel()]
        ).reshape(NTOT, 1)
        in_maps.append({"tm": tm, "ki16": ki16, "kiL1": kiL1, "kf": kf})
    return in_maps, NCH0, NCH1, n1last


# ---------------------------------------------------------------------------
# device kernel (raw Block bass: hand-placed semaphores, no TileContext)
# ---------------------------------------------------------------------------


def _build_nc(NCH0, NCH1, n1last):
    from contextlib import ExitStack

    from concourse import bacc, mybir
    from concourse.bass import IndirectOffsetOnAxis

    f32 = mybir.dt.float32
    i32 = mybir.dt.int32
    i16 = mybir.dt.int16
    Alu = mybir.AluOpType
    X = mybir.AxisListType.X
    Exp = mybir.ActivationFunctionType.Exp
    NCT = NCH0 + NCH1
    N16 = NCH0 * 128
    OHW = NCH0 * EB
    KFW = OHW + 4 * NCT + NCT * NH + 8
    tb = OHW                      # T region base
    eb_ = OHW + 4 * NCT           # E region base
    base = OHW + 4 * NCT + NCT * NH

    nc = bacc.Bacc()
    TM = nc.declare_dram_parameter("tm", [NTOT, 1], f32, isOutput=False)
    KI16 = nc.declare_dram_parameter("ki16", [128, N16 // 16], i16,
                                     isOutput=False)
    KIL1 = nc.declare_dram_parameter("kiL1", [128, NCH1], i32, isOutput=False)
    KF = nc.declare_dram_parameter("kf", [128, KFW], f32, isOutput=False)
    OUT = nc.declare_dram_parameter("out", [1, 1], f32, isOutput=True)

    with ExitStack() as ctx:
        e = ctx.enter_context
        ki16 = e(nc.sbuf_tensor("ki16_sb", [128, N16 // 16], i16))
        kiL1 = e(nc.sbuf_tensor("kiL1_sb", [128, NCH1], i32))
        kf = e(nc.sbuf_tensor("kf_sb", [128, KFW], f32))
        X0 = e(nc.sbuf_tensor("X0", [128, OHW], f32))
        M0 = e(nc.sbuf_tensor("M0", [128, OHW], f32))
        S = e(nc.sbuf_tensor("S", [128, NCT], f32))
        EST = e(nc.sbuf_tensor("EST", [128, NCT * NH], f32))
        avg = e(nc.sbuf_tensor("avg", [NH, 2], f32))
        u = e(nc.sbuf_tensor("u", [NH, 2], f32))
        avgsrc = e(nc.sbuf_tensor("avgsrc", [32, 64], f32))
        avgT = e(nc.sbuf_tensor("avgT", [32, 64], f32))
        d2 = e(nc.sbuf_tensor("d2", [NH, 64], f32))
        pm = e(nc.sbuf_tensor("pm", [NH, 64], f32))
        Z = e(nc.sbuf_tensor("Z", [NH, 4], f32))
        ones = e(nc.sbuf_tensor("ones", [NH, 1], f32))
        warm = e(nc.sbuf_tensor("warm", [1, 1], f32))
        fin = e(nc.sbuf_tensor("fin", [1, 4], f32))
        res = e(nc.sbuf_tensor("res", [1, 1], f32))
        ps_st = e(nc.psum_tensor("ps_st", [NH, 4], f32))
        ps_f = e(nc.psum_tensor("ps_f", [1, 4], f32))

        d_k16 = e(nc.semaphore("d_k16"))
        d_kL1 = e(nc.semaphore("d_kL1"))
        d_kf = e(nc.semaphore("d_kf"))
        g16 = e(nc.semaphore("g16"))
        gs = [e(nc.semaphore(f"gs{c}")) for c in range(NCH1)]
        vdone = e(nc.semaphore("vdone"))
        a_exp = e(nc.semaphore("a_exp"))
        p_st = e(nc.semaphore("p_st"))
        p_f = e(nc.semaphore("p_f"))
        d_out = e(nc.semaphore("d_out"))

        block = e(nc.Block())
        M = {}

        @block.vector
        def _(vector):
            n = 0

            def op(r, key=None):
                nonlocal n
                r.then_inc(vdone, 1)
                n += 1
                if key:
                    M[key] = n
                return n

            def wt(k):
                vector.wait_ge(vdone, k)

            op(vector.memset(avgsrc[:], BIG))
            op(vector.memset(ones[:], 1.0))
            op(vector.memset(warm[:], 0.0), "warm")
            if n1last < 128:
                # whole column (partition slices must be quadrant-aligned);
                # the last gather round orders itself after this via vdone
                op(vector.memset(S[:, NCT - 1 : NCT], 0.0), "sz")
            vector.wait_ge(d_kf, 16)
            # level-0 extraction: one-hot multiply + segmented reduce
            vector.wait_ge(g16, 16)
            a = op(vector.tensor_tensor(
                out=M0[:], in0=X0[:], in1=kf[:, 0:OHW], op=Alu.mult))
            wt(a)
            op(vector.reduce_sum(
                out=S[:, 0:NCH0],
                in_=M0[:].rearrange("p (c e) -> p c e", e=EB), axis=X))
            # per chunk: rhs cols 2:4 = S*[L0,L1]; lhsT = E*S (independent)
            for c in range(NCT):
                if c >= NCH0:
                    vector.wait_ge(gs[c - NCH0], 16)
                op(vector.tensor_tensor(
                    out=kf[:, tb + 4 * c + 2 : tb + 4 * c + 4].rearrange(
                        "p (o l) -> p o l", o=1),
                    in0=S[:, c : c + 1].to_broadcast([128, 1, 2]),
                    in1=kf[:, tb + 4 * c : tb + 4 * c + 2].rearrange(
                        "p (o l) -> p o l", o=1),
                    op=Alu.mult))
                op(vector.tensor_tensor(
                    out=EST[:, c * NH : (c + 1) * NH].rearrange(
                        "p (o h) -> p o h", o=1),
                    in0=S[:, c : c + 1].to_broadcast([128, 1, NH]),
                    in1=kf[:, eb_ + c * NH : eb_ + (c + 1) * NH
                           ].rearrange("p (o h) -> p o h", o=1),
                    op=Alu.mult), f"T{c}")
            # stats landed in psum: bcast+transpose drive the push tail;
            # avg comes right after, off the transpose path
            vector.wait_ge(p_st, 1)
            a = op(vector.tensor_tensor(
                out=avgsrc[0:NH, :].rearrange("p (l j) -> p l j", l=2),
                in0=ps_st[:, 0:2].to_broadcast([NH, 2, 32]),
                in1=kf[0:NH, base : base + 2].to_broadcast([NH, 2, 32]),
                op=Alu.mult))
            wt(a)
            op(vector.transpose(avgT[:], avgsrc[:]))
            op(vector.tensor_tensor(
                out=avg[:], in0=ps_st[:, 0:2],
                in1=kf[0:NH, base + 2 : base + 4], op=Alu.mult), "avg")
            # d2[i, l*32+j] = (avg_i - avg_j)^2: avgT holds -avg_j
            wt(M["avg"])
            a = op(vector.tensor_tensor(
                out=d2[:].rearrange("p (l j) -> p l j", l=2),
                in0=avg[:].rearrange("p (l o) -> p l o", o=1
                                     ).to_broadcast([NH, 2, 32]),
                in1=avgT[0:NH, :].rearrange("p (l j) -> p l j", l=2),
                op=Alu.add))
            wt(a)
            op(vector.tensor_tensor(
                out=d2[:], in0=d2[:], in1=d2[:], op=Alu.mult), "d2")
            # pull stats while ACT runs the push exp
            u_n = op(vector.tensor_tensor(
                out=u[:], in0=ps_st[:, 0:2], in1=avg[:], op=Alu.mult))
            wt(u_n)
            op(vector.tensor_tensor(
                out=Z[:, 2:4], in0=ps_st[:, 2:4], in1=u[:],
                op=Alu.subtract), "pull")
            # push row block sums once ACT finished the exp
            vector.wait_ge(a_exp, 1)
            op(vector.reduce_sum(
                out=Z[:, 0:2],
                in_=pm[:].rearrange("p (l j) -> p l j", l=2), axis=X),
                "push")
            # final weighted reduce of [push0, push1, pull0, pull1]
            vector.wait_ge(p_f, 1)
            a = op(vector.tensor_tensor(
                out=fin[:], in0=ps_f[:], in1=kf[0:1, base + 4 : base + 8],
                op=Alu.mult))
            wt(a)
            op(vector.reduce_sum(out=res[:], in_=fin[:], axis=X), "res")

        @block.sync
        def _(sync):
            # index DMAs alone on the sync HWDGE queue so the big kf
            # transfer (scalar queue) can't delay their completion sems
            sync.dma_start(out=ki16[:], in_=KI16[:]).then_inc(d_k16, 16)
            sync.dma_start(out=kiL1[:], in_=KIL1[:]).then_inc(d_kL1, 16)
            sync.wait_ge(vdone, M["res"])
            # completion sem but no wait: the block-exit DRAIN covers it
            sync.dma_start(out=OUT[:], in_=res[:]).then_inc(d_out, 16)

        @block.gpsimd
        def _(gpsimd):
            gpsimd.wait_ge(d_k16, 16)
            gpsimd.dma_gather(
                out_ap=X0[:].rearrange("p (c e) -> p c e", e=EB),
                in_ap=TM[:].rearrange("(v e) o -> v (e o)", e=EB),
                idxs_ap=ki16[:],
                num_idxs=N16,
                num_idxs_reg=N16,
                elem_size=EB,
            ).then_inc(g16, 16)
            gpsimd.wait_ge(d_kL1, 16)
            for c in range(NCH1):
                rows = 128 if c < NCH1 - 1 else n1last
                if rows < 128:
                    gpsimd.wait_ge(vdone, M["sz"])
                gpsimd.indirect_dma_start(
                    out=S[0:rows, NCH0 + c : NCH0 + c + 1],
                    out_offset=None,
                    in_=TM[:],
                    in_offset=IndirectOffsetOnAxis(
                        ap=kiL1[0:rows, c : c + 1], axis=0
                    ),
                ).then_inc(gs[c], 16)

        @block.scalar
        def _(scalar):
            scalar.dma_start(out=kf[:], in_=KF[:]).then_inc(d_kf, 16)
            scalar.wait_ge(vdone, M["warm"])
            scalar.activation(warm[:, 0:1], warm[:, 0:1], Exp)
            scalar.wait_ge(vdone, M["d2"])
            scalar.activation(
                pm[:], d2[:], Exp, scale=-0.5
            ).then_inc(a_exp, 1)

        @block.tensor
        def _(tensor):
            for c in range(NCT):
                tensor.wait_ge(vdone, M[f"T{c}"])
                mm = tensor.matmul(
                    ps_st[:],
                    lhsT=EST[:, c * NH : (c + 1) * NH],
                    rhs=kf[:, tb + 4 * c : tb + 4 * c + 4],
                    start=(c == 0),
                    stop=(c == NCT - 1),
                )
            mm.then_inc(p_st, 1)
            tensor.wait_ge(vdone, M["push"])
            tensor.matmul(
                ps_f[:], lhsT=ones[:], rhs=Z[:], start=True, stop=True
            ).then_inc(p_f, 1)

    nc.finalize()
    return nc


def _get_nc(NCH0, NCH1, n1last):
    key = (NCH0, NCH1, n1last)
    if key not in _CACHE:
        _CACHE[key] = _build_nc(NCH0, NCH1, n1last)
    return _CACHE[key]


def kernel(tag_maps0, tag_maps1, kps0, kps1):
    from concourse.bass_utils import run_bass_kernel_spmd

    in_maps, NCH0, NCH1, n1last = make_in_maps(
        tag_maps0, tag_maps1, kps0, kps1
    )
    nc = _get_nc(NCH0, NCH1, n1last)
    out = run_bass_kernel_spmd(nc, in_maps, core_ids=list(range(B)))
    return np.array(
        [np.asarray(out.results[b]["out"]).reshape(()) for b in range(B)],
        dtype=np.float32,
    )


# revision 11
# speedup vs baseline: 1.4356x; 1.4356x over previous
"""Distributed Trainium2 Bass kernel for the associative-embedding (AE) loss.

Problem: per image b (B=8), two tag maps (tm0 [J,256,256], tm1 [J,512,512]),
keypoints kps [NH, 3*J] (x, y, vis interleaved, NH=30 humans, J=17 joints).
Per level: gather tag values at (j, x, y), masked per-human mean, pull loss
(masked squared deviation / num_humans) + push loss (pairwise Gaussian of
means / num_humans^2).  Output: per-image loss [B] (sum over both levels).

Strategy: pure data-parallel over B across 8 NeuronCores (core b handles
image b).  The loss touches only the ~NH*2*J visible-keypoint elements of
the 22 MB of tag maps, so each core pulls exactly those scalars out of DRAM
via indirect (SWDGE) DMAs.  The HW indirect DMA emits one descriptor per
out-partition row (max 128 scattered elements per instruction, ~994 ns
fixed + 0.34 ns/descriptor of Q7 time, ~310 ns dispatch gap), so the host
packs ONLY the visible entries into ceil(V/128) chunks with the small
remainder chunk LAST so its data drains quickly (~1.2 us vs ~1.6 us for a
full 128-row chunk) after the final descriptor-generation burst.  Interior
pad slots keep index 0 (a harmless gather of tm[0], zeroed by the one-hot
maps); the partial last column is memset to zero, ordered only against the
final gather round.  Host-baked one-hot matrices let the tensor engine
reduce the chunk layout into per-human sufficient statistics via
lhsT = E*S, rhs = [L0, L1, S*L0, S*L1]; chunks are processed while later
gathers are still in flight.  The push loss uses a 32x32 DVE stream
transpose, two DVE ops for the squared pairwise differences, one Exp on
the scalar engine, a ones-vector matmul, and a 2-op weighted reduce; all
input-only quantities (masks, reciprocal counts, 1/num_humans weights) are
host-precomputed.  The output DMA carries a completion semaphore but no
wait: the block-exit DRAIN covers it.  Per-core output is one scalar; the
host stacks the 8 scalars into the final [B] vector.
"""

import numpy as np

B = 8
NH = 30
J = 17
H0 = W0 = 256
H1 = W1 = 512
N0 = J * H0 * W0
N1 = J * H1 * W1
NTOT = N0 + N1
BIG = 30.0                # pad rows -> exp(-(BIG+avg)^2/2) ~ 1e-170 ~ 0

_CACHE = {}


# ---------------------------------------------------------------------------
# host-side input prep: valid-packed gather indices + one-hot reduction maps
# ---------------------------------------------------------------------------


def make_in_maps(tag_maps0, tag_maps1, kps0, kps1):
    tag_maps0 = np.asarray(tag_maps0, dtype=np.float32)
    tag_maps1 = np.asarray(tag_maps1, dtype=np.float32)
    kps0 = np.asarray(kps0, dtype=np.int64)
    kps1 = np.asarray(kps1, dtype=np.int64)
    jr = np.arange(J)[None, :]
    per_img = []
    nv_max = 0
    for b in range(B):
        xs0, ys0, vs0 = kps0[b, :, 0::3], kps0[b, :, 1::3], kps0[b, :, 2::3]
        xs1, ys1, vs1 = kps1[b, :, 0::3], kps1[b, :, 1::3], kps1[b, :, 2::3]
        idx_hlj = np.concatenate(
            [jr * (H0 * W0) + xs0 * W0 + ys0,
             N0 + jr * (H1 * W1) + xs1 * W1 + ys1], axis=1
        )  # [30, 34] flat index per (human, level*J+joint)
        mask = np.concatenate([vs0 != 0, vs1 != 0], axis=1)  # [30, 34] bool
        hh, cc = np.nonzero(mask)     # valid entries: human, level*J+joint
        per_img.append((idx_hlj, mask, hh, cc))
        nv_max = max(nv_max, len(hh))
    NC = -(-nv_max // 128)            # chunks of <=128 descriptors
    n_last = nv_max - 128 * (NC - 1)  # last (remainder) chunk size

    in_maps = []
    for b in range(B):
        idx_hlj, mask, hh, cc = per_img[b]
        nv = len(hh)
        # pad slots keep index 0 (gather tm[0], zeroed by the one-hots)
        idxc = np.zeros((128, NC), np.int32)
        E = np.zeros((128, NC * NH), np.float32)
        T = np.zeros((128, 4 * NC), np.float32)  # [L0, L1, 0, 0] per chunk
        r = np.arange(nv)
        ch, p = r // 128, r % 128
        lvl = (cc // J).astype(np.int64)
        idxc[p, ch] = idx_hlj[hh, cc]
        E[p, ch * NH + hh] = 1.0
        T[p, ch * 4 + lvl] = 1.0

        cnt = np.stack([mask[:, :J].sum(1), mask[:, J:].sum(1)], 1).astype(
            np.float32
        )
        has = (cnt > 0).astype(np.float32)
        rdh = has / np.maximum(cnt, 1.0)
        P = 1.0 / has.sum(0)          # [2] 1/num_humans per level
        # kf layout [128, 4*NC + NC*NH + 8]:
        #   cols 0:4NC            T/rhs region (host: L0,L1,0,0; DVE: S*L)
        #   cols 4NC:4NC+NC*NH    E one-hot
        #   next 2: -rdh | next 2: rdh | next 4 (row 0): w4
        kf = np.zeros((128, 4 * NC + NC * NH + 8), np.float32)
        kf[:, 0 : 4 * NC] = T
        kf[:, 4 * NC : 4 * NC + NC * NH] = E
        base = 4 * NC + NC * NH
        kf[0:NH, base : base + 2] = -rdh
        kf[0:NH, base + 2 : base + 4] = rdh
        kf[0, base + 4 : base + 8] = [P[0] ** 2, P[1] ** 2, P[0], P[1]]
        tm = np.concatenate(
            [tag_maps0[b].ravel(), tag_maps1[b].ravel()]
        ).reshape(NTOT, 1)
        in_maps.append(
            {"tm": tm, "ki": np.ascontiguousarray(idxc[:, 1:]),
             "ki0": np.ascontiguousarray(idxc[:, 0:1]), "kf": kf}
        )
    return in_maps, NC, n_last


# ---------------------------------------------------------------------------
# device kernel (raw Block bass: hand-placed semaphores, no TileContext)
# ---------------------------------------------------------------------------


def _build_nc(NC, n_last):
    from contextlib import ExitStack

    from concourse import bacc, mybir
    from concourse.bass import IndirectOffsetOnAxis

    f32 = mybir.dt.float32
    i32 = mybir.dt.int32
    Alu = mybir.AluOpType
    X = mybir.AxisListType.X
    Exp = mybir.ActivationFunctionType.Exp
    KFW = 4 * NC + NC * NH + 8
    base = 4 * NC + NC * NH

    nc = bacc.Bacc()
    TM = nc.declare_dram_parameter("tm", [NTOT, 1], f32, isOutput=False)
    KI0 = nc.declare_dram_parameter("ki0", [128, 1], i32, isOutput=False)
    KI = nc.declare_dram_parameter("ki", [128, NC - 1], i32, isOutput=False)
    KF = nc.declare_dram_parameter("kf", [128, KFW], f32, isOutput=False)
    OUT = nc.declare_dram_parameter("out", [1, 1], f32, isOutput=True)

    with ExitStack() as ctx:
        e = ctx.enter_context
        ki0 = e(nc.sbuf_tensor("ki0_sb", [128, 1], i32))
        ki = e(nc.sbuf_tensor("ki_sb", [128, NC - 1], i32))
        kf = e(nc.sbuf_tensor("kf_sb", [128, KFW], f32))
        S = e(nc.sbuf_tensor("S", [128, NC], f32))
        EST = e(nc.sbuf_tensor("EST", [128, NC * NH], f32))
        avg = e(nc.sbuf_tensor("avg", [NH, 2], f32))
        u = e(nc.sbuf_tensor("u", [NH, 2], f32))
        avgsrc = e(nc.sbuf_tensor("avgsrc", [32, 64], f32))
        avgT = e(nc.sbuf_tensor("avgT", [32, 64], f32))
        d2 = e(nc.sbuf_tensor("d2", [NH, 64], f32))
        pm = e(nc.sbuf_tensor("pm", [NH, 64], f32))
        Z = e(nc.sbuf_tensor("Z", [NH, 4], f32))
        ones = e(nc.sbuf_tensor("ones", [NH, 1], f32))
        warm = e(nc.sbuf_tensor("warm", [1, 1], f32))
        fin = e(nc.sbuf_tensor("fin", [1, 4], f32))
        res = e(nc.sbuf_tensor("res", [1, 1], f32))
        ps_st = e(nc.psum_tensor("ps_st", [NH, 4], f32))
        ps_f = e(nc.psum_tensor("ps_f", [1, 4], f32))

        d_ki0 = e(nc.semaphore("d_ki0"))
        d_ki = e(nc.semaphore("d_ki"))
        d_kf = e(nc.semaphore("d_kf"))
        gs = [e(nc.semaphore(f"gs{c}")) for c in range(NC)]
        vdone = e(nc.semaphore("vdone"))
        a_exp = e(nc.semaphore("a_exp"))
        p_st = e(nc.semaphore("p_st"))
        p_f = e(nc.semaphore("p_f"))
        d_out = e(nc.semaphore("d_out"))

        block = e(nc.Block())
        M = {}

        @block.vector
        def _(vector):
            n = 0

            def op(r, key=None):
                nonlocal n
                r.then_inc(vdone, 1)
                n += 1
                if key:
                    M[key] = n
                return n

            def wt(k):
                vector.wait_ge(vdone, k)

            op(vector.memset(avgsrc[:], BIG))
            op(vector.memset(ones[:], 1.0))
            op(vector.memset(warm[:], 0.0), "warm")
            if n_last < 128:
                # whole column (partition slices must be quadrant-aligned);
                # the last gather round orders itself after this via vdone
                op(vector.memset(S[:, NC - 1 : NC], 0.0), "sz")
            vector.wait_ge(d_kf, 16)
            # per chunk: rhs cols 2:4 = S*[L0,L1]; lhsT = E*S (independent)
            for c in range(NC):
                vector.wait_ge(gs[c], 16)
                op(vector.tensor_tensor(
                    out=kf[:, 4 * c + 2 : 4 * c + 4].rearrange(
                        "p (o l) -> p o l", o=1),
                    in0=S[:, c : c + 1].to_broadcast([128, 1, 2]),
                    in1=kf[:, 4 * c : 4 * c + 2].rearrange(
                        "p (o l) -> p o l", o=1),
                    op=Alu.mult))
                op(vector.tensor_tensor(
                    out=EST[:, c * NH : (c + 1) * NH].rearrange(
                        "p (o h) -> p o h", o=1),
                    in0=S[:, c : c + 1].to_broadcast([128, 1, NH]),
                    in1=kf[:, 4 * NC + c * NH : 4 * NC + (c + 1) * NH
                           ].rearrange("p (o h) -> p o h", o=1),
                    op=Alu.mult), f"T{c}")
            # stats landed in psum: bcast+transpose drive the push tail;
            # avg comes right after, off the transpose path
            vector.wait_ge(p_st, 1)
            a = op(vector.tensor_tensor(
                out=avgsrc[0:NH, :].rearrange("p (l j) -> p l j", l=2),
                in0=ps_st[:, 0:2].to_broadcast([NH, 2, 32]),
                in1=kf[0:NH, base : base + 2].to_broadcast([NH, 2, 32]),
                op=Alu.mult))
            wt(a)
            op(vector.transpose(avgT[:], avgsrc[:]))
            op(vector.tensor_tensor(
                out=avg[:], in0=ps_st[:, 0:2],
                in1=kf[0:NH, base + 2 : base + 4], op=Alu.mult), "avg")
            # d2[i, l*32+j] = (avg_i - avg_j)^2: avgT holds -avg_j
            wt(M["avg"])
            a = op(vector.tensor_tensor(
                out=d2[:].rearrange("p (l j) -> p l j", l=2),
                in0=avg[:].rearrange("p (l o) -> p l o", o=1
                                     ).to_broadcast([NH, 2, 32]),
                in1=avgT[0:NH, :].rearrange("p (l j) -> p l j", l=2),
                op=Alu.add))
            wt(a)
            op(vector.tensor_tensor(
                out=d2[:], in0=d2[:], in1=d2[:], op=Alu.mult), "d2")
            # pull stats while ACT runs the push exp
            u_n = op(vector.tensor_tensor(
                out=u[:], in0=ps_st[:, 0:2], in1=avg[:], op=Alu.mult))
            wt(u_n)
            op(vector.tensor_tensor(
                out=Z[:, 2:4], in0=ps_st[:, 2:4], in1=u[:],
                op=Alu.subtract), "pull")
            # push row block sums once ACT finished the exp
            vector.wait_ge(a_exp, 1)
            op(vector.reduce_sum(
                out=Z[:, 0:2],
                in_=pm[:].rearrange("p (l j) -> p l j", l=2), axis=X),
                "push")
            # final weighted reduce of [push0, push1, pull0, pull1]
            vector.wait_ge(p_f, 1)
            a = op(vector.tensor_tensor(
                out=fin[:], in0=ps_f[:], in1=kf[0:1, base + 4 : base + 8],
                op=Alu.mult))
            wt(a)
            op(vector.reduce_sum(out=res[:], in_=fin[:], axis=X), "res")

        @block.sync
        def _(sync):
            # ki alone on the sync HWDGE queues so the big kf transfer
            # (issued by scalar) can't delay its completion semaphore;
            # chunk 0's 512B column first so the gather chain starts early
            sync.dma_start(out=ki0[:], in_=KI0[:]).then_inc(d_ki0, 16)
            sync.dma_start(out=ki[:], in_=KI[:]).then_inc(d_ki, 16)
            sync.wait_ge(vdone, M["res"])
            # completion sem but no wait: the block-exit DRAIN covers it
            sync.dma_start(out=OUT[:], in_=res[:]).then_inc(d_out, 16)

        @block.gpsimd
        def _(gpsimd):
            gpsimd.wait_ge(d_ki0, 16)
            gpsimd.indirect_dma_start(
                out=S[:, 0:1],
                out_offset=None,
                in_=TM[:],
                in_offset=IndirectOffsetOnAxis(ap=ki0[:], axis=0),
            ).then_inc(gs[0], 16)
            gpsimd.wait_ge(d_ki, 16)
            for c in range(1, NC):
                rows = 128 if c < NC - 1 else n_last
                if rows < 128:
                    gpsimd.wait_ge(vdone, M["sz"])
                gpsimd.indirect_dma_start(
                    out=S[0:rows, c : c + 1],
                    out_offset=None,
                    in_=TM[:],
                    in_offset=IndirectOffsetOnAxis(
                        ap=ki[0:rows, c - 1 : c], axis=0
                    ),
                ).then_inc(gs[c], 16)

        @block.scalar
        def _(scalar):
            scalar.dma_start(out=kf[:], in_=KF[:]).then_inc(d_kf, 16)
            scalar.wait_ge(vdone, M["warm"])
            scalar.activation(warm[:, 0:1], warm[:, 0:1], Exp)
            scalar.wait_ge(vdone, M["d2"])
            scalar.activation(
                pm[:], d2[:], Exp, scale=-0.5
            ).then_inc(a_exp, 1)

        @block.tensor
        def _(tensor):
            for c in range(NC):
                tensor.wait_ge(vdone, M[f"T{c}"])
                mm = tensor.matmul(
                    ps_st[:],
                    lhsT=EST[:, c * NH : (c + 1) * NH],
                    rhs=kf[:, 4 * c : 4 * c + 4],
                    start=(c == 0),
                    stop=(c == NC - 1),
                )
            mm.then_inc(p_st, 1)
            tensor.wait_ge(vdone, M["push"])
            tensor.matmul(
                ps_f[:], lhsT=ones[:], rhs=Z[:], start=True, stop=True
            ).then_inc(p_f, 1)

    nc.finalize()
    return nc


def _get_nc(NC, n_last):
    key = (NC, n_last)
    if key not in _CACHE:
        _CACHE[key] = _build_nc(NC, n_last)
    return _CACHE[key]


def kernel(tag_maps0, tag_maps1, kps0, kps1):
    from concourse.bass_utils import run_bass_kernel_spmd

    in_maps, NC, n_last = make_in_maps(tag_maps0, tag_maps1, kps0, kps1)
    nc = _get_nc(NC, n_last)
    out = run_bass_kernel_spmd(nc, in_maps, core_ids=list(range(B)))
    return np.array(
        [np.asarray(out.results[b]["out"]).reshape(()) for b in range(B)],
        dtype=np.float32,
    )
